# revision 6
# baseline (speedup 1.0000x reference)
"""Trainium2 Bass kernel for the CgpHmmCell forward log-likelihood.

Reference computation: a 3-state HMM forward pass over T=2048 steps on
B=4096 sequences with 4-symbol one-hot observations, returning the final
log-likelihood per sequence ([B, 1] f32).

Mathematically the per-step normalization in the reference telescopes:
    loglik_T = log( 1^T  M_{T-1} ... M_1  alpha_0 ),
    M_t = diag(E_t) A^T,  E_t[s] = Bmat[s, tok_t].
M_t has block structure  [[pi, 0], [q, R]] (state 0 never re-entered), so a
product of M's is parametrized by (pi: scalar, q: 2-vec, R: 2x2) = 7 numbers.
The kernel computes per-time-chunk products of M's with time chunks laid out
across the 128 SBUF partitions (batch in the free dimension), renormalizes
once per chunk, transposes to batch-major with the TensorE, and folds the 128
chunk maps with a binary tree along the free dimension.

Sharding: pure batch data parallelism, 512 sequences per NeuronCore x 8.
"""

import numpy as np

_B, _T, _ALPHA = 4096, 2048, 4
_NCORES = 8
_BPC = _B // _NCORES        # 512 sequences per core
_C, _L = 128, 16            # time chunks (partitions) x steps per chunk
_NSUB, _BSUB = 4, 128       # batch sub-tiles per core


def _host_params(transition_kernel, emission_kernel):
    t = np.asarray(transition_kernel, np.float32)
    sig = lambda z: np.float32(1.0) / (np.float32(1.0) + np.exp(-z, dtype=np.float32))
    a00 = float(sig(np.float32(1.0) - 2 * t[0]))
    a11 = float(sig(np.float32(1.0) - 2 * t[1]))
    a21 = float(sig(2 * t[2] - np.float32(1.0)))
    ek = np.asarray(emission_kernel, np.float32)
    e = np.exp(ek - ek.max(axis=-1, keepdims=True), dtype=np.float32)
    Bmat = (e / e.sum(axis=-1, keepdims=True)).astype(np.float32)
    return a00, a11, a21, Bmat


def _build(a00, a11, a21, Bmat, linearize=False):
    import dataclasses
    from contextlib import ExitStack

    import concourse.bacc as bacc
    import concourse.mybir as mybir
    import concourse.tile as tile
    from concourse import masks

    f32, bf16 = mybir.dt.float32, mybir.dt.bfloat16
    MUL, ADD, MAX = mybir.AluOpType.mult, mybir.AluOpType.add, mybir.AluOpType.max
    Ln = mybir.ActivationFunctionType.Ln

    g = a11 / a21
    h = (1.0 - a11) / (1.0 - a21)
    jj = (1.0 - a00) / a21
    # lincomb coefficients producing the pre-scaled emission planes:
    #   Ehat0 = a00*E0, Et1 = a21*E1, Et2 = (1-a21)*E2
    c0 = [float(a00 * Bmat[0, a]) for a in range(_ALPHA)]
    c1 = [float(a21 * Bmat[1, a]) for a in range(_ALPHA)]
    c2 = [float((1.0 - a21) * Bmat[2, a]) for a in range(_ALPHA)]

    def bc3(ap):
        """Broadcast an AP over a leading free dim of 3 (the packed columns)."""
        return dataclasses.replace(ap, ap=[ap.ap[0], [0, 3], *ap.ap[1:]])

    nc = bacc.Bacc("TRN2", target_bir_lowering=False, debug=False)
    x_t = nc.dram_tensor("x", [_BPC, _T, _ALPHA], f32, kind="ExternalInput")
    out_t = nc.dram_tensor("out", [_BPC, 1], f32, kind="ExternalOutput")

    with tile.TileContext(nc, linearize=linearize) as tc, ExitStack() as ctx:
        pool = ctx.enter_context(tc.tile_pool(name="main", bufs=1))
        xpool = ctx.enter_context(tc.tile_pool(name="xin", bufs=2))
        apool = ctx.enter_context(tc.tile_pool(name="acc", bufs=2))
        ppool = ctx.enter_context(tc.tile_pool(name="ps", bufs=4, space="PSUM"))

        ident = pool.tile([128, 128], f32, tag="ident")
        masks.make_identity(nc, ident[:, :])

        # persistent emission-plane buffers [chunk, sub, b, k] (k innermost)
        Eb = [
            pool.tile([128, _NSUB, _BSUB, _L], bf16, tag=f"Eb{i}", name=f"Eb{i}")
            for i in range(3)
        ]

        # ---- phase 1a: stream x, build emission planes --------------------
        for sub in range(_NSUB):
            X = xpool.tile([128, _BSUB, _L, _ALPHA], f32, tag="X")
            src = x_t[sub * _BSUB : (sub + 1) * _BSUB].rearrange(
                "b (c k) a -> c b k a", c=_C, k=_L
            )
            nc.sync.dma_start(X[:, :, :, :], src)
            for ei, cc in ((0, c0), (1, c1), (2, c2)):
                acc = apool.tile([128, _BSUB, _L], f32, tag="acc")
                nc.vector.tensor_scalar_mul(acc[:, :, :], X[:, :, :, 0], cc[0])
                for a in (1, 2):
                    nc.vector.scalar_tensor_tensor(
                        acc[:, :, :], X[:, :, :, a], cc[a], acc[:, :, :], MUL, ADD
                    )
                nc.vector.scalar_tensor_tensor(
                    Eb[ei][:, sub, :, :],
                    X[:, :, :, 3],
                    cc[3],
                    acc[:, :, :],
                    MUL,
                    ADD,
                )

        # ---- phase 1b: init chunk maps from the k=0 step ------------------
        # P1 = [q1, R11, R12], P2 = [q2, R21, R22], each [128, 3, sub, b]
        P1 = pool.tile([128, 3, _NSUB, _BSUB], f32, tag="P1")
        P2 = pool.tile([128, 3, _NSUB, _BSUB], f32, tag="P2")
        s1 = pool.tile([128, 3, _NSUB, _BSUB], f32, tag="s1")
        s2 = pool.tile([128, 3, _NSUB, _BSUB], f32, tag="s2")
        pi = pool.tile([128, _NSUB, _BSUB], f32, tag="pi")

        e0k0 = Eb[0][:, :, :, 0]
        e1k0 = Eb[1][:, :, :, 0]
        e2k0 = Eb[2][:, :, :, 0]
        nc.scalar.copy(pi[:, :, :], e0k0)
        nc.vector.tensor_scalar_mul(P1[:, 0, :, :], e1k0, jj)   # q1
        nc.vector.tensor_scalar_mul(P1[:, 1, :, :], e1k0, g)    # R11
        nc.scalar.copy(P1[:, 2, :, :], e1k0)                    # R12
        nc.vector.memset(P2[:, 0, :, :], 0.0)                   # q2
        nc.vector.tensor_scalar_mul(P2[:, 1, :, :], e2k0, h)    # R21
        nc.scalar.copy(P2[:, 2, :, :], e2k0)                    # R22
        # chunk 0 carries only the t=0 emission as a rank-1 map
        nc.vector.tensor_scalar_mul(pi[0:1, :, :], Eb[0][0:1, :, :, 0], 1.0 / a00)
        nc.vector.memset(P1[0:1, :, :, :], 0.0)
        nc.vector.memset(P2[0:1, :, :, :], 0.0)

        # ---- phase 1c: the 15 remaining steps of each chunk ---------------
        for k in range(1, _L):
            e0k = Eb[0][:, :, :, k]
            e1k = Eb[1][:, :, :, k]
            e2k = Eb[2][:, :, :, k]
            nc.vector.scalar_tensor_tensor(
                s1[:, :, :, :], P1[:, :, :, :], g, P2[:, :, :, :], MUL, ADD
            )
            nc.vector.scalar_tensor_tensor(
                s1[:, 0, :, :], pi[:, :, :], jj, s1[:, 0, :, :], MUL, ADD
            )
            nc.vector.scalar_tensor_tensor(
                s2[:, :, :, :], P1[:, :, :, :], h, P2[:, :, :, :], MUL, ADD
            )
            nc.vector.tensor_tensor(P1[:, :, :, :], s1[:, :, :, :], bc3(e1k), MUL)
            nc.vector.tensor_tensor(P2[:, :, :, :], s2[:, :, :, :], bc3(e2k), MUL)
            nc.vector.tensor_tensor(pi[:, :, :], pi[:, :, :], e0k, MUL)

        # ---- phase 1d: per-chunk renormalization --------------------------
        m = pool.tile([128, _NSUB, _BSUB], f32, tag="m")
        inv = pool.tile([128, _NSUB, _BSUB], f32, tag="inv")
        lsig = pool.tile([128, _NSUB, _BSUB], f32, tag="lsig")
        nc.vector.tensor_tensor(m[:, :, :], P1[:, 0, :, :], P1[:, 1, :, :], MAX)
        nc.vector.tensor_tensor(m[:, :, :], m[:, :, :], P1[:, 2, :, :], MAX)
        for col in range(3):
            nc.vector.tensor_tensor(m[:, :, :], m[:, :, :], P2[:, col, :, :], MAX)
        nc.vector.tensor_tensor(m[:, :, :], m[:, :, :], pi[:, :, :], MAX)
        nc.vector.reciprocal(inv[:, :, :], m[:, :, :])
        nc.scalar.activation(lsig[:, :, :], m[:, :, :], Ln)
        nc.vector.tensor_tensor(P1[:, :, :, :], P1[:, :, :, :], bc3(inv[:, :, :]), MUL)
        nc.vector.tensor_tensor(P2[:, :, :, :], P2[:, :, :, :], bc3(inv[:, :, :]), MUL)
        nc.vector.tensor_tensor(pi[:, :, :], pi[:, :, :], inv[:, :, :], MUL)

        # ---- phase 2: transpose to batch-major [b, plane, sub, chunk] -----
        # plane order: 0:pi 1:q1 2:q2 3:R11 4:R12 5:R21 6:R22 7:lsig
        planes = [
            pi[:, :, :],
            P1[:, 0, :, :],
            P2[:, 0, :, :],
            P1[:, 1, :, :],
            P1[:, 2, :, :],
            P2[:, 1, :, :],
            P2[:, 2, :, :],
            lsig[:, :, :],
        ]
        TA = pool.tile([128, 8, _NSUB, _C], f32, tag="TA")
        TB = pool.tile([128, 8, _NSUB, _C // 2], f32, tag="TB")
        for pl in range(8):
            for bg in range(_NSUB):
                ps = ppool.tile([128, 128], f32, tag="ps")
                nc.tensor.matmul(
                    ps[:, :], planes[pl][:, bg, :], ident[:, :], is_transpose=True
                )
                nc.scalar.copy(TA[:, pl, bg, :], ps[:, :])

        # ---- phase 3: binary-tree fold of the 128 chunk maps --------------
        t1 = pool.tile([128, _NSUB, _C // 2], f32, tag="t1")
        t2 = pool.tile([128, _NSUB, _C // 2], f32, tag="t2")
        SRC, DST = TA, TB
        W = _C // 2
        while W >= 1:
            E = lambda pl: SRC[:, pl, :, 0 : 2 * W : 2]
            O = lambda pl: SRC[:, pl, :, 1 : 2 * W : 2]
            ts1 = t1[:, :, 0:W]
            ts2 = t2[:, :, 0:W]
            D = lambda pl: DST[:, pl, :, 0:W]
            nc.vector.tensor_tensor(D(0), E(0), O(0), MUL)       # pi
            nc.vector.tensor_tensor(D(7), E(7), O(7), ADD)       # log-scale
            for qi, (r1, r2) in ((1, (3, 4)), (2, (5, 6))):
                nc.vector.tensor_tensor(ts1, O(qi), E(0), MUL)
                nc.vector.tensor_tensor(ts2, O(r1), E(1), MUL)
                nc.vector.tensor_tensor(ts1, ts1, ts2, ADD)
                nc.vector.tensor_tensor(ts2, O(r2), E(2), MUL)
                nc.vector.tensor_tensor(D(qi), ts1, ts2, ADD)
            for ri, (ra, rb, ca, cb) in (
                (3, (3, 4, 3, 5)),
                (4, (3, 4, 4, 6)),
                (5, (5, 6, 3, 5)),
                (6, (5, 6, 4, 6)),
            ):
                nc.vector.tensor_tensor(ts1, O(ra), E(ca), MUL)
                nc.vector.tensor_tensor(ts2, O(rb), E(cb), MUL)
                nc.vector.tensor_tensor(D(ri), ts1, ts2, ADD)
            if W == 16:
                # mid-tree renorm: keeps magnitudes inside ACT-Ln's domain
                mm = ts1[:, :, :]
                nc.vector.tensor_tensor(mm, D(0), D(1), MAX)
                for pl in range(2, 7):
                    nc.vector.tensor_tensor(mm, mm, D(pl), MAX)
                iv = ts2[:, :, :]
                nc.vector.reciprocal(iv, mm)
                for pl in range(7):
                    nc.vector.tensor_tensor(D(pl), D(pl), iv, MUL)
                lnm = pool.tile([128, _NSUB, 16], f32, tag="lnm")
                nc.scalar.activation(lnm[:, :, :], mm, Ln)
                nc.vector.tensor_tensor(D(7), D(7), lnm[:, :, :], ADD)
            SRC, DST = DST, SRC
            W //= 2

        # after the loop SRC holds the width-1 result
        F = pool.tile([128, _NSUB, 1], f32, tag="F")
        G = pool.tile([128, _NSUB, 1], f32, tag="G")
        res = pool.tile([128, _NSUB, 1], f32, tag="res")
        nc.vector.tensor_tensor(F[:, :, :], SRC[:, 1, :, 0:1], SRC[:, 2, :, 0:1], ADD)
        nc.vector.tensor_tensor(F[:, :, :], F[:, :, :], SRC[:, 0, :, 0:1], ADD)
        nc.scalar.activation(G[:, :, :], F[:, :, :], Ln)
        nc.vector.tensor_tensor(res[:, :, :], G[:, :, :], SRC[:, 7, :, 0:1], ADD)
        dst = out_t[:, :].rearrange("(s p) one -> p s one", p=128)
        nc.sync.dma_start(dst, res[:, :, :])

    nc.compile()
    return nc


def kernel(x, transition_kernel, emission_kernel):
    a00, a11, a21, Bmat = _host_params(transition_kernel, emission_kernel)
    nc = _build(a00, a11, a21, Bmat)

    from concourse import bass_utils

    x = np.ascontiguousarray(np.asarray(x, np.float32))
    in_maps = [
        {"x": x[i * _BPC : (i + 1) * _BPC]} for i in range(_NCORES)
    ]
    r = bass_utils.run_bass_kernel_spmd(nc, in_maps, core_ids=list(range(_NCORES)))
    return np.concatenate([m["out"] for m in r.results], axis=0).astype(np.float32)


if __name__ == "__main__":
    rng = np.random.default_rng(0)
    toks = rng.integers(0, 4, (_B, _T))
    x = np.eye(4, dtype=np.float32)[toks]
    tk = rng.normal(size=3).astype(np.float32) * 0.05
    ek = rng.normal(size=(3, 4)).astype(np.float32) * 0.05
    print(kernel(x, tk, ek)[:4, 0])
